# revision 13
# baseline (speedup 1.0000x reference)
"""Distributed multi-head attention kernel for one TRN2 chip (8 NeuronCores).

Problem: x[2,2048,1024] -> qkv -> 16-head attention -> out proj, f32 I/O.

Sharding: 8 cores = 2 batches x 4 head-groups (4 heads each).
Core c: batch b=c//4, head group g=c%4 (heads 4g..4g+3).

Structure (v3 — exp-saturated pipeline with fine-grained filler):
 - ScalarE does ONLY the softmax exps (the irreducible ~16.8M elem/core
   bottleneck, ~147us floor); all other tensor work (qkT, v, PV, proj)
   is split into ~1us "filler" units whose emission is interleaved into
   the attention chunk loops, so the Tile scheduler packs them into the
   PE's slack at fine grain.  This keeps the PE HAM clock-gate warm
   (idle never exceeds the ~3.4us re-throttle window) and ScalarE fed.
 - Tile dependency tracking is by EMISSION order, so producers are
   always emitted before consumers; scheduling preference is set by
   priority (pre-emitted v chunks 0-7 are priority-demoted gap fillers).
 - The final (pair1, qtile3) combo is split into two half-width
   sub-combos with two small AllGathers so the last gather+projection
   pipeline under compute (shorter serial tail).
 - A dummy 16B AllGather at T=0 absorbs the ~20-30us collectives
   bootstrap barrier into the input-DMA window; a tiny exp preloads the
   ACT exp table; input DMAs spread across SP/ACT HWDGE + gpsimd SWDGE.

Device algorithm (per core), all matmuls bf16 with f32 PSUM accumulate:
 1) qkT = wqk.T @ xT   [512,2048]  (q weights/bias pre-scaled by
    1/sqrt(dh) on host; bias added during DVE eviction)
 2) v_aug = xT.T @ w_v [2048, 4*65] (bias via DVE; per-head ones column
    interleaved so the PV matmul also emits softmax denominators)
 3) per (pair, q-range): per chunk-pair: sT = k @ qT (row-tiled head
    pair), p = exp(sT) on ScalarE (PSUM->SBUF bf16), oT_aug += v_aug.T
    @ p (M=65: row 64 = denominator); normalize oT by 1/denom on DVE.
 4) per q-range: AllGather o of both pairs (256 rows); proj with w_proj
    chunks stationary: out.T[of, q] += wp.T @ o_full; + bias; DMA out
    [256, 2048] transposed (host un-transposes).
"""

import os
import sys
import types
from collections import deque
import numpy as np
import ml_dtypes

import concourse.bass as bass
import concourse.mybir as mybir
import concourse.bacc as bacc
import concourse.tile as tile
from concourse.bass_utils import run_bass_kernel_spmd

BF16 = mybir.dt.bfloat16
F32 = mybir.dt.float32

B, N, D = 2, 2048, 1024
H, DH = 16, 64
SCALE = DH ** -0.5

P = 128                 # partitions
NT = 512                # token free-dim tile
KC = N // P             # 16 k-token chunks
QT = N // NT            # 4 q tiles
DC = D // P             # 8 d_model chunks
HPC = 4                 # heads per core
OF = HPC * DH           # 256 o-features per core
VW = HPC * (DH + 1)     # v_aug width (260): per head [v(64) | ones(1)]

CORE_IDS = list(range(8))
GROUPS = [[0, 1, 2, 3], [4, 5, 6, 7]]
LAST_RESULTS = None


def _install_ntff_shim():
    """Provide antenv.axon_hooks (absent from this image's antenv stub) so
    run_bass_kernel_spmd(trace=True) can reach the NTFF profiler in
    libaxon_pjrt.so. Only needed when profiling."""
    if "antenv.axon_hooks" in sys.modules:
        return
    try:
        from trn_agent_boot.trn_boot import _ntff_profile_via_ctypes
        hook = _ntff_profile_via_ctypes("/opt/axon/libaxon_pjrt.so")
    except Exception:
        hook = None
    mod = types.ModuleType("antenv.axon_hooks")
    mod._hook = hook
    mod.get_axon_ntff_profile_hook = lambda: mod._hook
    mod.set_axon_ntff_profile_hook = lambda h: setattr(mod, "_hook", h)
    sys.modules["antenv.axon_hooks"] = mod


def build_nc():
    nc = bacc.Bacc("TRN2", target_bir_lowering=False, debug=False, num_devices=8)

    xt_ext = nc.dram_tensor("xt", [D, N], BF16, kind="ExternalInput")
    wqk_ext = nc.dram_tensor("wqk", [D, 2 * OF], BF16, kind="ExternalInput")
    wv_ext = nc.dram_tensor("wv", [D, OF], BF16, kind="ExternalInput")
    bqk_ext = nc.dram_tensor("bqk", [P, 4], F32, kind="ExternalInput")
    bv_ext = nc.dram_tensor("bv", [1, OF], F32, kind="ExternalInput")
    wp_ext = nc.dram_tensor("wp", [D, OF], BF16, kind="ExternalInput")
    bp_ext = nc.dram_tensor("bp", [P, 2], F32, kind="ExternalInput")
    # transposed output [of, q]; host transposes back
    out_ext = nc.dram_tensor("out", [OF, N], F32, kind="ExternalOutput")

    # AllGather bounce buffers per q-range (qt 0,1,2 full; qt3 in halves —
    # ranges 3,4 — so the tail gather pipelines; range 5 = full qt3, used
    # only by pair 0's attention which feeds both half buffers).
    # Separate tensors so Tile's whole-tensor DRAM dep tracking never
    # falsely serializes.  Ranges: (q0, qw).
    QRANGES = [(0, NT), (NT, NT), (2 * NT, NT),
               (3 * NT, NT // 2), (3 * NT + NT // 2, NT // 2),
               (3 * NT, NT)]
    ag_in = [nc.dram_tensor(f"ag_in_{i}", [2 * P, qw], BF16)
             for i, (q0, qw) in enumerate(QRANGES[:5])]
    ag_out = [nc.dram_tensor(f"ag_out_{i}", [8 * P, qw], BF16)
              for i, (q0, qw) in enumerate(QRANGES[:5])]
    # dummy collective to absorb the bootstrap barrier at T=0
    agw_in = nc.dram_tensor("agw_in", [1, 16], BF16)
    agw_out = nc.dram_tensor("agw_out", [4, 16], BF16)

    with tile.TileContext(nc) as tc:
        with (
            tc.tile_pool(name="xt_pool", bufs=1) as xt_pool,
            tc.tile_pool(name="w_pool", bufs=1) as w_pool,
            tc.tile_pool(name="qk_pool", bufs=1) as qk_pool,
            tc.tile_pool(name="v_pool", bufs=1) as v_pool,
            tc.tile_pool(name="const_pool", bufs=1) as const_pool,
            tc.tile_pool(name="pt_pool", bufs=12) as pt_pool,
            tc.tile_pool(name="o_pool", bufs=4) as o_pool,
            tc.tile_pool(name="nrm_pool", bufs=2) as nrm_pool,
            tc.tile_pool(name="ofull_pool", bufs=10) as ofull_pool,
            tc.tile_pool(name="pr_pool", bufs=4) as pr_pool,
            tc.tile_pool(name="sw_pool", bufs=2, space="PSUM") as sw_pool,
            tc.tile_pool(name="po_pool", bufs=1, space="PSUM") as po_pool,
            tc.tile_pool(name="aux_pool", bufs=2, space="PSUM") as aux_pool,
        ):
            # ---- T=0 warmups ------------------------------------------------
            nc.gpsimd.collective_compute(
                "AllGather", mybir.AluOpType.bypass,
                replica_groups=GROUPS,
                ins=[agw_in.ap().opt()],
                outs=[agw_out.ap().opt()])
            warm_in = const_pool.tile([1, 16], F32)
            warm_out = const_pool.tile([1, 16], F32)
            nc.vector.memset(warm_in[:], 0.0)
            nc.scalar.activation(
                warm_out[:], warm_in[:], mybir.ActivationFunctionType.Exp)

            # ---- input loads: wqk+xt on the two HWDGE queues (gate phase B);
            #      wv/wp/consts on gpsimd SWDGE queues.
            xt_t, wqk_t, wv_t = [], [], []
            for k in range(DC):
                eng = nc.sync if (k % 2 == 0) else nc.scalar
                tw = w_pool.tile([P, 2 * OF], BF16, name=f"wqk{k}")
                eng.dma_start(tw[:], wqk_ext[k * P:(k + 1) * P, :])
                wqk_t.append(tw)
                eng2 = nc.scalar if (k % 2 == 0) else nc.sync
                tx = xt_pool.tile([P, N], BF16, name=f"xt{k}")
                eng2.dma_start(tx[:], xt_ext[k * P:(k + 1) * P, :])
                xt_t.append(tx)

            bqk_sb = const_pool.tile([P, 4], F32)
            nc.gpsimd.dma_start(bqk_sb[:], bqk_ext[:])
            bv_row = const_pool.tile([1, OF], F32)
            nc.gpsimd.dma_start(bv_row[:], bv_ext[:])
            bp_sb = const_pool.tile([P, 2], F32)
            nc.gpsimd.dma_start(bp_sb[:], bp_ext[:])
            bv_bc = const_pool.tile([P, OF], F32)
            nc.gpsimd.partition_broadcast(bv_bc[:], bv_row[:])

            for k in range(DC):
                t = w_pool.tile([P, OF], BF16, name=f"wv{k}")
                nc.gpsimd.dma_start(t[:], wv_ext[k * P:(k + 1) * P, :])
                wv_t.append(t)
            wp_t = {}
            for pair in range(2):
                for g in range(4):
                    t = w_pool.tile([P, OF], BF16, name=f"wp{pair}{g}")
                    r0 = 256 * g + 128 * pair
                    nc.gpsimd.dma_start(t[:], wp_ext[r0:r0 + P, :])
                    wp_t[(pair, g)] = t

            # ---- filler framework ------------------------------------------
            # Units of ~0.5-1.5us of TensorE work, drained (emitted) between
            # attention chunk iterations so the scheduler can slot them into
            # PE slack at fine grain.
            FILL = deque()

            def drain(n):
                for _ in range(n):
                    if FILL:
                        FILL.popleft()()

            # ---- phase B: qkT = wqk.T @ xT -> 4 tiles [128, 2048] bf16 ------
            # m=0: q heads 0-1, m=1: q heads 2-3, m=2: k heads 0-1, m=3: k 2-3
            qk_sb = [qk_pool.tile([P, N], BF16, name=f"qk{m}") for m in range(4)]

            def qk_mtile_units(m):
                """qkT m-tile as 4 units of (np, k-half): LDW-amortized
                n-pair inner loop, DVE eviction with bias on the last."""
                units = []
                for np_ in range(2):
                    state = {}

                    def make(np_, kh, state):
                        def u():
                            if kh == 0:
                                state["ps"] = [
                                    aux_pool.tile([P, NT], F32, name="aux")
                                    for _ in range(2)]
                            for k in range(4 * kh, 4 * kh + 4):
                                for j in range(2):
                                    n = 2 * np_ + j
                                    nc.tensor.matmul(
                                        state["ps"][j][:],
                                        wqk_t[k][:, m * P:(m + 1) * P],
                                        xt_t[k][:, n * NT:(n + 1) * NT],
                                        start=(k == 0), stop=(k == DC - 1))
                            if kh == 1:
                                for j in range(2):
                                    n = 2 * np_ + j
                                    nc.vector.tensor_scalar_add(
                                        qk_sb[m][:, n * NT:(n + 1) * NT],
                                        state["ps"][j][:],
                                        bqk_sb[:, m:m + 1])
                        return u
                    units.append(make(np_, 0, state))
                    units.append(make(np_, 1, state))
                return units

            def qk_mtile(m):
                for u in qk_mtile_units(m):
                    u()

            # ---- phase C: v_aug [2048, 260] bf16 (ones interleaved) ---------
            v_sb = [v_pool.tile([P, VW], BF16, name=f"v{t}") for t in range(KC)]

            def v_units(t):
                state = {}

                def ua():
                    state["ps"] = aux_pool.tile([P, OF], F32, name="aux")
                    for k in range(4):
                        nc.tensor.matmul(
                            state["ps"][:], xt_t[k][:, t * P:(t + 1) * P],
                            wv_t[k][:], start=(k == 0), stop=False)

                def ub():
                    ps = state["ps"]
                    for k in range(4, 8):
                        nc.tensor.matmul(
                            ps[:], xt_t[k][:, t * P:(t + 1) * P], wv_t[k][:],
                            start=False, stop=(k == DC - 1))
                    vdst = v_sb[t][:, :].rearrange("p (h c) -> p h c", c=DH + 1)
                    nc.vector.tensor_add(
                        vdst[:, :, 0:DH],
                        ps[:, :].rearrange("p (h c) -> p h c", c=DH),
                        bv_bc[:, :].rearrange("p (h c) -> p h c", c=DH))
                    nc.vector.memset(vdst[:, :, DH:DH + 1], 1.0)
                return [ua, ub]

            # ---- phase D: attention per (pair, q-range) ---------------------
            def attn_range(p, ri, hook=None):
                q0, qw = QRANGES[ri]
                kt = qk_sb[2 + p]
                qt_ = qk_sb[p]
                hA, hB = 2 * p, 2 * p + 1
                qs = slice(q0, q0 + qw)
                po0 = po_pool.tile([DH + 1, qw], F32, name="po0")
                po1 = po_pool.tile([DH + 1, qw], F32, name="po1")
                for c2 in range(KC // 2):
                    if hook:
                        hook(c2)
                    c, c1 = 2 * c2, 2 * c2 + 1
                    cs = slice(c * P, (c + 1) * P)
                    cs1 = slice(c1 * P, (c1 + 1) * P)
                    st, sp = (c == 0), (c1 == KC - 1)
                    swA = sw_pool.tile([P, 2 * qw], F32, name="sw")
                    nc.tensor.matmul(swA[:, 0:qw], kt[0:64, cs],
                                     qt_[0:64, qs], tile_position=(0, 0),
                                     start=True, stop=True)
                    nc.tensor.matmul(swA[:, qw:2 * qw], kt[0:64, cs1],
                                     qt_[0:64, qs], tile_position=(0, 0),
                                     start=True, stop=True)
                    ptA = pt_pool.tile([P, 2 * qw], BF16, name="pt")
                    nc.scalar.activation(
                        ptA[:], swA[:], mybir.ActivationFunctionType.Exp)
                    swB = sw_pool.tile([P, 2 * qw], F32, name="sw")
                    nc.tensor.matmul(swB[:, 0:qw], kt[64:128, cs],
                                     qt_[64:128, qs], tile_position=(64, 0),
                                     start=True, stop=True)
                    nc.tensor.matmul(swB[:, qw:2 * qw], kt[64:128, cs1],
                                     qt_[64:128, qs], tile_position=(64, 0),
                                     start=True, stop=True)
                    ptB = pt_pool.tile([P, 2 * qw], BF16, name="pt")
                    nc.scalar.activation(
                        ptB[:], swB[:], mybir.ActivationFunctionType.Exp)
                    nc.tensor.matmul(
                        po0[:], v_sb[c][:, hA * (DH + 1):(hA + 1) * (DH + 1)],
                        ptA[:, 0:qw], start=st, stop=False)
                    nc.tensor.matmul(
                        po0[:], v_sb[c1][:, hA * (DH + 1):(hA + 1) * (DH + 1)],
                        ptA[:, qw:2 * qw], start=False, stop=sp)
                    nc.tensor.matmul(
                        po1[:], v_sb[c][:, hB * (DH + 1):(hB + 1) * (DH + 1)],
                        ptB[:, 0:qw], start=st, stop=False)
                    nc.tensor.matmul(
                        po1[:], v_sb[c1][:, hB * (DH + 1):(hB + 1) * (DH + 1)],
                        ptB[:, qw:2 * qw], start=False, stop=sp)
                # normalize by 1/denominator (psum row 64, per q token)
                ot = o_pool.tile([P, qw], BF16, name="o")
                for hi, po in ((0, po0), (1, po1)):
                    d = nrm_pool.tile([1, qw], F32, name="d")
                    nc.vector.tensor_copy(d[0:1, :], po[64:65, :])
                    r = nrm_pool.tile([1, qw], F32, name="r")
                    scr = nrm_pool.tile([1, qw], F32, name="scr")
                    nc.vector.reciprocal_approx_accurate(
                        r[0:1, :], d[0:1, :], scr[0:1, :])
                    rb = nrm_pool.tile([64, qw], F32, name="rb")
                    nc.gpsimd.partition_broadcast(rb[0:64, :], r[0:1, :])
                    nc.vector.tensor_mul(
                        ot[64 * hi:64 * (hi + 1), :], po[0:64, :], rb[0:64, :])
                if ri == 5:
                    # pair-0 qt3 (full width) feeds both half-range buffers
                    nc.sync.dma_start(ag_in[3][0:P, :], ot[:, 0:NT // 2])
                    nc.sync.dma_start(ag_in[4][0:P, :], ot[:, NT // 2:NT])
                else:
                    nc.sync.dma_start(ag_in[ri][p * P:(p + 1) * P, :], ot[:])

            def ag_range(ri):
                nc.gpsimd.collective_compute(
                    "AllGather", mybir.AluOpType.bypass,
                    replica_groups=GROUPS,
                    ins=[ag_in[ri].ap().opt()],
                    outs=[ag_out[ri].ap().opt()])

            # ---- proj per q-range: wp stationary, out.T[of, q] --------------
            def proj_units(ri):
                q0, qw = QRANGES[ri]
                ofull = []

                def load():
                    for cg in range(8):
                        t = ofull_pool.tile([P, qw], BF16, name="ofull")
                        nc.sync.dma_start(
                            t[:], ag_out[ri][cg * P:(cg + 1) * P, :])
                        ofull.append(t)
                units = [load]
                for h in range(2):
                    state = {}

                    def mk(h=h, half=0, state=state):
                        def u():
                            if half == 0:
                                state["ps"] = aux_pool.tile(
                                    [P, qw], F32, name="aux")
                            for cg in range(4 * half, 4 * half + 4):
                                g, pair = cg // 2, cg % 2
                                nc.tensor.matmul(
                                    state["ps"][:],
                                    wp_t[(pair, g)][:, h * P:(h + 1) * P],
                                    ofull[cg][:],
                                    start=(cg == 0), stop=(cg == 7))
                            if half == 1:
                                pr = pr_pool.tile([P, qw], F32, name="pr")
                                nc.vector.tensor_scalar_add(
                                    pr[:], state["ps"][:], bp_sb[:, h:h + 1])
                                nc.sync.dma_start(
                                    out_ext[h * P:(h + 1) * P, q0:q0 + qw],
                                    pr[:])
                        return u
                    units.append(mk(h, 0, state))
                    units.append(mk(h, 1, state))
                return units

            # ---- emission ---------------------------------------------------
            # (order also defines dependencies: producers before consumers)
            qk_mtile(2)
            qk_mtile(0)
            # v chunks 0-7: pre-emitted, priority-demoted -> pure gap filler
            with tc.high_priority(offset=-1_000_000):
                for t in range(8):
                    for u in v_units(t):
                        u()
            # v chunks 8-15 hooked into combo (0,0): chunk 8+c2 emitted at
            # iteration c2 (PV of chunk 2c2+1 needs chunk <= 8+c2: ok)
            v_late = deque()
            for t in range(8, KC):
                v_late.extend(v_units(t))

            def hook_v(c2):
                for _ in range(2):
                    if v_late:
                        v_late.popleft()()
                drain(1)

            for u in qk_mtile_units(1):
                FILL.append(u)
            for u in qk_mtile_units(3):
                FILL.append(u)
            attn_range(0, 0, hook=hook_v)
            attn_range(0, 1, hook=lambda c2: drain(1))
            attn_range(1, 0, hook=lambda c2: drain(1))
            ag_range(0)
            for u in proj_units(0):
                FILL.append(u)
            attn_range(0, 2, hook=lambda c2: drain(1))
            attn_range(1, 1, hook=lambda c2: drain(1))
            ag_range(1)
            for u in proj_units(1):
                FILL.append(u)
            attn_range(0, 5, hook=lambda c2: drain(1))
            attn_range(1, 2, hook=lambda c2: drain(1))
            ag_range(2)
            for u in proj_units(2):
                FILL.append(u)
            attn_range(1, 3, hook=lambda c2: drain(2))
            ag_range(3)
            attn_range(1, 4, hook=lambda c2: drain(2))
            ag_range(4)
            drain(len(FILL))
            for u in proj_units(3):
                u()
            for u in proj_units(4):
                u()

    nc.compile()
    return nc


_NC_CACHE = None


def _get_nc():
    global _NC_CACHE
    if _NC_CACHE is None:
        _NC_CACHE = build_nc()
    return _NC_CACHE


def _bf16(a):
    return np.ascontiguousarray(a.astype(ml_dtypes.bfloat16))


def kernel(x, w_qkv, b_qkv, w_proj, b_proj):
    global LAST_RESULTS
    x = np.asarray(x, dtype=np.float32)
    w_qkv = np.asarray(w_qkv, dtype=np.float32)
    b_qkv = np.asarray(b_qkv, dtype=np.float32)
    w_proj = np.asarray(w_proj, dtype=np.float32)
    b_proj = np.asarray(b_proj, dtype=np.float32)

    nc = _get_nc()

    in_maps = []
    for c in CORE_IDS:
        b, g = c // 4, c % 4
        cs = slice(g * OF, (g + 1) * OF)   # feature cols of this head group
        wq = w_qkv[:, 0 * D:1 * D][:, cs] * SCALE
        wk = w_qkv[:, 1 * D:2 * D][:, cs]
        wv = w_qkv[:, 2 * D:3 * D][:, cs]
        bq = b_qkv[0 * D:1 * D][cs] * SCALE
        bk = b_qkv[1 * D:2 * D][cs]
        bqk = np.concatenate([bq, bk]).reshape(4, P).T.copy()  # [128, 4]
        in_maps.append({
            "xt": _bf16(x[b].T),
            "wqk": _bf16(np.concatenate([wq, wk], axis=1)),
            "wv": _bf16(wv),
            "bqk": np.ascontiguousarray(bqk, dtype=np.float32),
            "bv": np.ascontiguousarray(
                b_qkv[2 * D + g * OF:2 * D + (g + 1) * OF].reshape(1, OF)),
            "wp": _bf16(w_proj[:, cs]),
            "bp": np.ascontiguousarray(
                b_proj[cs].reshape(2, P).T, dtype=np.float32),
        })

    trace = bool(os.environ.get("KERNEL_TRACE"))
    if trace:
        _install_ntff_shim()
    LAST_RESULTS = run_bass_kernel_spmd(
        nc, in_maps, CORE_IDS, trace=trace)

    out = np.empty((B, N, D), dtype=np.float32)
    for c in CORE_IDS:
        b, g = c // 4, c % 4
        out[b, :, g * OF:(g + 1) * OF] = LAST_RESULTS.results[c]["out"].T
    return out


# revision 21
# speedup vs baseline: 1.0117x; 1.0117x over previous
"""Distributed multi-head attention kernel for one TRN2 chip (8 NeuronCores).

Problem: x[2,2048,1024] -> qkv -> 16-head attention -> out proj, f32 I/O.

Sharding: 8 cores = 2 batches x 4 head-groups (4 heads each).
Core c: batch b=c//4, head group g=c%4 (heads 4g..4g+3).

Structure (v3 — exp-saturated pipeline with fine-grained filler):
 - ScalarE does ONLY the softmax exps (the irreducible ~16.8M elem/core
   bottleneck, ~147us floor); all other tensor work (qkT, v, PV, proj)
   is split into ~1us "filler" units whose emission is interleaved into
   the attention chunk loops, so the Tile scheduler packs them into the
   PE's slack at fine grain.  This keeps the PE HAM clock-gate warm
   (idle never exceeds the ~3.4us re-throttle window) and ScalarE fed.
 - Tile dependency tracking is by EMISSION order, so producers are
   always emitted before consumers; scheduling preference is set by
   priority (pre-emitted v chunks 0-7 are priority-demoted gap fillers).
 - The final (pair1, qtile3) combo is split into two half-width
   sub-combos with two small AllGathers so the last gather+projection
   pipeline under compute (shorter serial tail).
 - A dummy 16B AllGather at T=0 absorbs the ~20-30us collectives
   bootstrap barrier into the input-DMA window; a tiny exp preloads the
   ACT exp table; input DMAs spread across SP/ACT HWDGE + gpsimd SWDGE.

Device algorithm (per core), all matmuls bf16 with f32 PSUM accumulate:
 1) qkT = wqk.T @ xT   [512,2048]  (q weights/bias pre-scaled by
    1/sqrt(dh) on host; bias added during DVE eviction)
 2) v_aug = xT.T @ w_v [2048, 4*65] (bias via DVE; per-head ones column
    interleaved so the PV matmul also emits softmax denominators)
 3) per (pair, q-range): per chunk-pair: sT = k @ qT (row-tiled head
    pair), p = exp(sT) on ScalarE (PSUM->SBUF bf16), oT_aug += v_aug.T
    @ p (M=65: row 64 = denominator); normalize oT by 1/denom on DVE.
 4) per q-range: AllGather o of both pairs (256 rows); proj with w_proj
    chunks stationary: out.T[of, q] += wp.T @ o_full; + bias; DMA out
    [256, 2048] transposed (host un-transposes).
"""

import os
import sys
import types
from collections import deque
import numpy as np
import ml_dtypes

import concourse.bass as bass
import concourse.mybir as mybir
import concourse.bacc as bacc
import concourse.tile as tile
from concourse.bass_utils import run_bass_kernel_spmd

BF16 = mybir.dt.bfloat16
F32 = mybir.dt.float32

B, N, D = 2, 2048, 1024
H, DH = 16, 64
SCALE = DH ** -0.5

P = 128                 # partitions
NT = 512                # token free-dim tile
KC = N // P             # 16 k-token chunks
QT = N // NT            # 4 q tiles
DC = D // P             # 8 d_model chunks
HPC = 4                 # heads per core
OF = HPC * DH           # 256 o-features per core
VW = HPC * (DH + 1)     # v_aug width (260): per head [v(64) | ones(1)]

CORE_IDS = list(range(8))
GROUPS = [[0, 1, 2, 3], [4, 5, 6, 7]]
LAST_RESULTS = None


def _install_ntff_shim():
    """Provide antenv.axon_hooks (absent from this image's antenv stub) so
    run_bass_kernel_spmd(trace=True) can reach the NTFF profiler in
    libaxon_pjrt.so. Only needed when profiling."""
    if "antenv.axon_hooks" in sys.modules:
        return
    try:
        from trn_agent_boot.trn_boot import _ntff_profile_via_ctypes
        hook = _ntff_profile_via_ctypes("/opt/axon/libaxon_pjrt.so")
    except Exception:
        hook = None
    mod = types.ModuleType("antenv.axon_hooks")
    mod._hook = hook
    mod.get_axon_ntff_profile_hook = lambda: mod._hook
    mod.set_axon_ntff_profile_hook = lambda h: setattr(mod, "_hook", h)
    sys.modules["antenv.axon_hooks"] = mod


def build_nc():
    nc = bacc.Bacc("TRN2", target_bir_lowering=False, debug=False, num_devices=8)

    xt_ext = nc.dram_tensor("xt", [D, N], BF16, kind="ExternalInput")
    wqk_ext = nc.dram_tensor("wqk", [D, 2 * OF], BF16, kind="ExternalInput")
    wv_ext = nc.dram_tensor("wv", [D, OF], BF16, kind="ExternalInput")
    bqk_ext = nc.dram_tensor("bqk", [P, 4], F32, kind="ExternalInput")
    bv_ext = nc.dram_tensor("bv", [1, OF], F32, kind="ExternalInput")
    wp_ext = nc.dram_tensor("wp", [D, OF], BF16, kind="ExternalInput")
    bp_ext = nc.dram_tensor("bp", [P, 2], F32, kind="ExternalInput")
    # transposed output [of, q]; host transposes back
    out_ext = nc.dram_tensor("out", [OF, N], F32, kind="ExternalOutput")

    # AllGather bounce buffers per q-range (qt 0,1,2 full; qt3 in halves —
    # ranges 3,4 — so the tail gather pipelines; range 5 = full qt3, used
    # only by pair 0's attention which feeds both half buffers).
    # Separate tensors so Tile's whole-tensor DRAM dep tracking never
    # falsely serializes.  Ranges: (q0, qw).
    QRANGES = [(0, NT), (NT, NT), (2 * NT, NT),
               (3 * NT, NT // 2), (3 * NT + NT // 2, NT // 2),
               (3 * NT, NT)]
    ag_in = [nc.dram_tensor(f"ag_in_{i}", [2 * P, qw], BF16)
             for i, (q0, qw) in enumerate(QRANGES[:5])]
    ag_out = [nc.dram_tensor(f"ag_out_{i}", [8 * P, qw], BF16)
              for i, (q0, qw) in enumerate(QRANGES[:5])]
    # dummy collective to absorb the bootstrap barrier at T=0
    agw_in = nc.dram_tensor("agw_in", [1, 16], BF16)
    agw_out = nc.dram_tensor("agw_out", [4, 16], BF16)

    with tile.TileContext(nc) as tc:
        with (
            tc.tile_pool(name="xt_pool", bufs=1) as xt_pool,
            tc.tile_pool(name="w_pool", bufs=1) as w_pool,
            tc.tile_pool(name="qk_pool", bufs=1) as qk_pool,
            tc.tile_pool(name="v_pool", bufs=1) as v_pool,
            tc.tile_pool(name="const_pool", bufs=1) as const_pool,
            tc.tile_pool(name="pt_pool", bufs=12) as pt_pool,
            tc.tile_pool(name="o_pool", bufs=4) as o_pool,
            tc.tile_pool(name="nrm_pool", bufs=2) as nrm_pool,
            tc.tile_pool(name="ofull_pool", bufs=10) as ofull_pool,
            tc.tile_pool(name="pr_pool", bufs=4) as pr_pool,
            tc.tile_pool(name="sw_pool", bufs=2, space="PSUM") as sw_pool,
            tc.tile_pool(name="po_pool", bufs=1, space="PSUM") as po_pool,
            tc.tile_pool(name="aux_pool", bufs=2, space="PSUM") as aux_pool,
        ):
            # ---- T=0 warmups ------------------------------------------------
            nc.gpsimd.collective_compute(
                "AllGather", mybir.AluOpType.bypass,
                replica_groups=GROUPS,
                ins=[agw_in.ap().opt()],
                outs=[agw_out.ap().opt()])
            warm_in = const_pool.tile([1, 16], F32)
            warm_out = const_pool.tile([1, 16], F32)
            nc.vector.memset(warm_in[:], 0.0)
            nc.scalar.activation(
                warm_out[:], warm_in[:], mybir.ActivationFunctionType.Exp)

            # ---- input loads: wqk+xt on the two HWDGE queues (gate phase B);
            #      wv/wp/consts on gpsimd SWDGE queues.
            xt_t, wqk_t, wv_t = [], [], []
            for k in range(DC):
                eng = nc.sync if (k % 2 == 0) else nc.scalar
                tw = w_pool.tile([P, 2 * OF], BF16, name=f"wqk{k}")
                eng.dma_start(tw[:], wqk_ext[k * P:(k + 1) * P, :])
                wqk_t.append(tw)
                eng2 = nc.scalar if (k % 2 == 0) else nc.sync
                tx = xt_pool.tile([P, N], BF16, name=f"xt{k}")
                eng2.dma_start(tx[:], xt_ext[k * P:(k + 1) * P, :])
                xt_t.append(tx)

            bqk_sb = const_pool.tile([P, 4], F32)
            nc.sync.dma_start(bqk_sb[:], bqk_ext[:])
            bv_row = const_pool.tile([1, OF], F32)
            nc.sync.dma_start(bv_row[:], bv_ext[:])
            bp_sb = const_pool.tile([P, 2], F32)
            nc.scalar.dma_start(bp_sb[:], bp_ext[:])
            bv_bc = const_pool.tile([P, OF], F32)
            nc.gpsimd.partition_broadcast(bv_bc[:], bv_row[:])
            # ones row for the tensor-engine denominator broadcast
            ones_sb = const_pool.tile([1, 64], BF16)
            nc.vector.memset(ones_sb[:], 1.0)

            for k in range(DC):
                eng = nc.sync if (k % 2 == 0) else nc.scalar
                t = w_pool.tile([P, OF], BF16, name=f"wv{k}")
                eng.dma_start(t[:], wv_ext[k * P:(k + 1) * P, :])
                wv_t.append(t)
            wp_t = {}
            for pair in range(2):
                for g in range(4):
                    eng = nc.sync if ((pair + g) % 2 == 0) else nc.scalar
                    t = w_pool.tile([P, OF], BF16, name=f"wp{pair}{g}")
                    r0 = 256 * g + 128 * pair
                    eng.dma_start(t[:], wp_ext[r0:r0 + P, :])
                    wp_t[(pair, g)] = t

            # ---- filler framework ------------------------------------------
            # Units of ~0.5-1.5us of TensorE work, drained (emitted) between
            # attention chunk iterations so the scheduler can slot them into
            # PE slack at fine grain.
            FILL = deque()

            def drain(n):
                for _ in range(n):
                    if FILL:
                        FILL.popleft()()

            # ---- phase B: qkT = wqk.T @ xT -> 4 tiles [128, 2048] bf16 ------
            # m=0: q heads 0-1, m=1: q heads 2-3, m=2: k heads 0-1, m=3: k 2-3
            qk_sb = [qk_pool.tile([P, N], BF16, name=f"qk{m}") for m in range(4)]

            def qk_mtile_units(m):
                """qkT m-tile as 4 units of (np, k-half): LDW-amortized
                n-pair inner loop, DVE eviction with bias on the last."""
                units = []
                for np_ in range(2):
                    state = {}

                    def make(np_, kh, state):
                        def u():
                            if kh == 0:
                                state["ps"] = [
                                    aux_pool.tile([P, NT], F32, name="aux")
                                    for _ in range(2)]
                            for k in range(4 * kh, 4 * kh + 4):
                                for j in range(2):
                                    n = 2 * np_ + j
                                    nc.tensor.matmul(
                                        state["ps"][j][:],
                                        wqk_t[k][:, m * P:(m + 1) * P],
                                        xt_t[k][:, n * NT:(n + 1) * NT],
                                        start=(k == 0), stop=(k == DC - 1))
                            if kh == 1:
                                for j in range(2):
                                    n = 2 * np_ + j
                                    nc.vector.tensor_scalar_add(
                                        qk_sb[m][:, n * NT:(n + 1) * NT],
                                        state["ps"][j][:],
                                        bqk_sb[:, m:m + 1])
                        return u
                    units.append(make(np_, 0, state))
                    units.append(make(np_, 1, state))
                return units

            def qk_mtile(m):
                for u in qk_mtile_units(m):
                    u()

            # ---- phase C: v_aug [2048, 260] bf16 (ones interleaved) ---------
            v_sb = [v_pool.tile([P, VW], BF16, name=f"v{t}") for t in range(KC)]

            def v_units(t):
                state = {}

                def ua():
                    state["ps"] = aux_pool.tile([P, OF], F32, name="aux")
                    for k in range(4):
                        nc.tensor.matmul(
                            state["ps"][:], xt_t[k][:, t * P:(t + 1) * P],
                            wv_t[k][:], start=(k == 0), stop=False)

                def ub():
                    ps = state["ps"]
                    for k in range(4, 8):
                        nc.tensor.matmul(
                            ps[:], xt_t[k][:, t * P:(t + 1) * P], wv_t[k][:],
                            start=False, stop=(k == DC - 1))
                    vdst = v_sb[t][:, :].rearrange("p (h c) -> p h c", c=DH + 1)
                    nc.vector.tensor_add(
                        vdst[:, :, 0:DH],
                        ps[:, :].rearrange("p (h c) -> p h c", c=DH),
                        bv_bc[:, :].rearrange("p (h c) -> p h c", c=DH))
                    nc.vector.memset(vdst[:, :, DH:DH + 1], 1.0)
                return [ua, ub]

            # ---- phase D: attention per (pair, q-range) ---------------------
            def attn_range(p, ri, hook=None):
                q0, qw = QRANGES[ri]
                kt = qk_sb[2 + p]
                qt_ = qk_sb[p]
                hA, hB = 2 * p, 2 * p + 1
                qs = slice(q0, q0 + qw)
                po0 = po_pool.tile([DH + 1, qw], F32, name="po0")
                po1 = po_pool.tile([DH + 1, qw], F32, name="po1")
                for c2 in range(KC // 2):
                    if hook:
                        hook(c2)
                    c, c1 = 2 * c2, 2 * c2 + 1
                    cs = slice(c * P, (c + 1) * P)
                    cs1 = slice(c1 * P, (c1 + 1) * P)
                    st, sp = (c == 0), (c1 == KC - 1)
                    swA = sw_pool.tile([P, 2 * qw], F32, name="sw")
                    nc.tensor.matmul(swA[:, 0:qw], kt[0:64, cs],
                                     qt_[0:64, qs], tile_position=(0, 0),
                                     start=True, stop=True)
                    nc.tensor.matmul(swA[:, qw:2 * qw], kt[0:64, cs1],
                                     qt_[0:64, qs], tile_position=(0, 0),
                                     start=True, stop=True)
                    ptA = pt_pool.tile([P, 2 * qw], BF16, name="pt")
                    nc.scalar.activation(
                        ptA[:], swA[:], mybir.ActivationFunctionType.Exp)
                    swB = sw_pool.tile([P, 2 * qw], F32, name="sw")
                    nc.tensor.matmul(swB[:, 0:qw], kt[64:128, cs],
                                     qt_[64:128, qs], tile_position=(64, 0),
                                     start=True, stop=True)
                    nc.tensor.matmul(swB[:, qw:2 * qw], kt[64:128, cs1],
                                     qt_[64:128, qs], tile_position=(64, 0),
                                     start=True, stop=True)
                    ptB = pt_pool.tile([P, 2 * qw], BF16, name="pt")
                    nc.scalar.activation(
                        ptB[:], swB[:], mybir.ActivationFunctionType.Exp)
                    nc.tensor.matmul(
                        po0[:], v_sb[c][:, hA * (DH + 1):(hA + 1) * (DH + 1)],
                        ptA[:, 0:qw], start=st, stop=False)
                    nc.tensor.matmul(
                        po0[:], v_sb[c1][:, hA * (DH + 1):(hA + 1) * (DH + 1)],
                        ptA[:, qw:2 * qw], start=False, stop=sp)
                    nc.tensor.matmul(
                        po1[:], v_sb[c][:, hB * (DH + 1):(hB + 1) * (DH + 1)],
                        ptB[:, 0:qw], start=st, stop=False)
                    nc.tensor.matmul(
                        po1[:], v_sb[c1][:, hB * (DH + 1):(hB + 1) * (DH + 1)],
                        ptB[:, qw:2 * qw], start=False, stop=sp)
                # normalize by 1/denominator (psum row 64, per q token)
                ot = o_pool.tile([P, qw], BF16, name="o")
                for hi, po in ((0, po0), (1, po1)):
                    d = nrm_pool.tile([1, qw], F32, name="d")
                    nc.vector.tensor_copy(d[0:1, :], po[64:65, :])
                    r = nrm_pool.tile([1, qw], F32, name="r")
                    scr = nrm_pool.tile([1, qw], F32, name="scr")
                    nc.vector.reciprocal_approx_accurate(
                        r[0:1, :], d[0:1, :], scr[0:1, :])
                    rb = nrm_pool.tile([64, qw], F32, name="rb")
                    nc.gpsimd.partition_broadcast(rb[0:64, :], r[0:1, :])
                    nc.vector.tensor_mul(
                        ot[64 * hi:64 * (hi + 1), :], po[0:64, :], rb[0:64, :])
                if ri == 5:
                    # pair-0 qt3 (full width) feeds both half-range buffers
                    nc.sync.dma_start(ag_in[3][0:P, :], ot[:, 0:NT // 2])
                    nc.sync.dma_start(ag_in[4][0:P, :], ot[:, NT // 2:NT])
                else:
                    nc.sync.dma_start(ag_in[ri][p * P:(p + 1) * P, :], ot[:])

            def ag_range(ri):
                nc.gpsimd.collective_compute(
                    "AllGather", mybir.AluOpType.bypass,
                    replica_groups=GROUPS,
                    ins=[ag_in[ri].ap().opt()],
                    outs=[ag_out[ri].ap().opt()])

            # ---- proj per q-range: wp stationary, out.T[of, q] --------------
            def proj_units(ri):
                q0, qw = QRANGES[ri]
                ofull = []

                def load():
                    # on gpsimd (SWDGE): its wait on the AllGather semaphore
                    # must not head-of-line-block the sync queue's o->ag_in
                    # stores; behind it on gpsimd are only later collective
                    # triggers, which are serial with this AG anyway.
                    for cg in range(8):
                        t = ofull_pool.tile([P, qw], BF16, name="ofull")
                        nc.gpsimd.dma_start(
                            t[:], ag_out[ri][cg * P:(cg + 1) * P, :])
                        ofull.append(t)
                units = [load]
                for h in range(2):
                    state = {}

                    def mk(h=h, half=0, state=state):
                        def u():
                            if half == 0:
                                state["ps"] = aux_pool.tile(
                                    [P, qw], F32, name="aux")
                            for cg in range(4 * half, 4 * half + 4):
                                g, pair = cg // 2, cg % 2
                                nc.tensor.matmul(
                                    state["ps"][:],
                                    wp_t[(pair, g)][:, h * P:(h + 1) * P],
                                    ofull[cg][:],
                                    start=(cg == 0), stop=(cg == 7))
                            if half == 1:
                                pr = pr_pool.tile([P, qw], F32, name="pr")
                                nc.vector.tensor_scalar_add(
                                    pr[:], state["ps"][:], bp_sb[:, h:h + 1])
                                nc.sync.dma_start(
                                    out_ext[h * P:(h + 1) * P, q0:q0 + qw],
                                    pr[:])
                        return u
                    units.append(mk(h, 0, state))
                    units.append(mk(h, 1, state))
                return units

            # ---- emission ---------------------------------------------------
            # (order also defines dependencies: producers before consumers)
            qk_mtile(2)
            qk_mtile(0)
            # v chunks 0-7: pre-emitted, priority-demoted -> pure gap filler
            with tc.high_priority(offset=-1_000_000):
                for t in range(8):
                    for u in v_units(t):
                        u()
            # v chunks 8-15 hooked into combo (0,0): chunk 8+c2 emitted at
            # iteration c2 (PV of chunk 2c2+1 needs chunk <= 8+c2: ok)
            v_late = deque()
            for t in range(8, KC):
                v_late.extend(v_units(t))

            def hook_v(c2):
                for _ in range(2):
                    if v_late:
                        v_late.popleft()()
                drain(1)

            for u in qk_mtile_units(1):
                FILL.append(u)
            for u in qk_mtile_units(3):
                FILL.append(u)
            attn_range(0, 0, hook=hook_v)
            attn_range(0, 1, hook=lambda c2: drain(1))
            attn_range(1, 0, hook=lambda c2: drain(1))
            ag_range(0)
            attn_range(0, 2, hook=lambda c2: drain(1))
            for u in proj_units(0):
                FILL.append(u)
            attn_range(1, 1, hook=lambda c2: drain(1))
            ag_range(1)
            attn_range(0, 5, hook=lambda c2: drain(1))
            for u in proj_units(1):
                FILL.append(u)
            attn_range(1, 2, hook=lambda c2: drain(1))
            ag_range(2)
            attn_range(1, 3, hook=lambda c2: drain(2))
            for u in proj_units(2):
                FILL.append(u)
            ag_range(3)
            attn_range(1, 4, hook=lambda c2: drain(2))
            ag_range(4)
            drain(len(FILL))
            for u in proj_units(3):
                u()
            for u in proj_units(4):
                u()

    nc.compile()
    return nc


_NC_CACHE = None


def _get_nc():
    global _NC_CACHE
    if _NC_CACHE is None:
        _NC_CACHE = build_nc()
    return _NC_CACHE


def _bf16(a):
    return np.ascontiguousarray(a.astype(ml_dtypes.bfloat16))


def kernel(x, w_qkv, b_qkv, w_proj, b_proj):
    global LAST_RESULTS
    x = np.asarray(x, dtype=np.float32)
    w_qkv = np.asarray(w_qkv, dtype=np.float32)
    b_qkv = np.asarray(b_qkv, dtype=np.float32)
    w_proj = np.asarray(w_proj, dtype=np.float32)
    b_proj = np.asarray(b_proj, dtype=np.float32)

    nc = _get_nc()

    in_maps = []
    for c in CORE_IDS:
        b, g = c // 4, c % 4
        cs = slice(g * OF, (g + 1) * OF)   # feature cols of this head group
        wq = w_qkv[:, 0 * D:1 * D][:, cs] * SCALE
        wk = w_qkv[:, 1 * D:2 * D][:, cs]
        wv = w_qkv[:, 2 * D:3 * D][:, cs]
        bq = b_qkv[0 * D:1 * D][cs] * SCALE
        bk = b_qkv[1 * D:2 * D][cs]
        bqk = np.concatenate([bq, bk]).reshape(4, P).T.copy()  # [128, 4]
        in_maps.append({
            "xt": _bf16(x[b].T),
            "wqk": _bf16(np.concatenate([wq, wk], axis=1)),
            "wv": _bf16(wv),
            "bqk": np.ascontiguousarray(bqk, dtype=np.float32),
            "bv": np.ascontiguousarray(
                b_qkv[2 * D + g * OF:2 * D + (g + 1) * OF].reshape(1, OF)),
            "wp": _bf16(w_proj[:, cs]),
            "bp": np.ascontiguousarray(
                b_proj[cs].reshape(2, P).T, dtype=np.float32),
        })

    trace = bool(os.environ.get("KERNEL_TRACE"))
    if trace:
        _install_ntff_shim()
    LAST_RESULTS = run_bass_kernel_spmd(
        nc, in_maps, CORE_IDS, trace=trace)

    out = np.empty((B, N, D), dtype=np.float32)
    for c in CORE_IDS:
        b, g = c // 4, c % 4
        out[b, :, g * OF:(g + 1) * OF] = LAST_RESULTS.results[c]["out"].T
    return out
